# revision 38
# baseline (speedup 1.0000x reference)
"""Single-head attention (B=4, T=4096, C=1024, H=64) on 8 trn2 NeuronCores.

Sharding: 8 shards = (batch b, query-half h).  Each core receives x[b]
pre-transposed to xT [C=1024, T=4096] in bf16; for h==1 the T columns are
rotated by 2048 so that "this core's" 2048 queries are always columns 0:2048
(softmax is permutation-invariant over keys).  SPMD program identical on all
cores, no rank logic.

Per-core kernel (flash-attention style; [T,T] scores never touch DRAM):
  phase 1: stream xT bf16; PE computes KV^T = [Wk|Wv]^T x^T (bf16, rows 0:64
           K^T, 64:128 V^T) over all 8 key blocks and Q^T = Wq^T x^T over
           the 4 query blocks; V^T tiles are PE-transposed to V[s,64] bf16
           with a ones column (softmax denominator falls out of attn@V).
  phase 2: per 128-key tile st and 1024-query unit: PE scoresT [128,1024] =
           K^T{64,128}.T @ Q^T{64,1024}; ACT exp(s/8) -> bf16; PE
           outT[65,1024] += V_aug.T @ ex accumulated over 32 key tiles.
  epilogue: raw outT slabs [65, 512] (64 value rows + exp-sum row) DMA out;
           host does the divide + transpose (off the HW critical path).

Scheduling: phase 2 is software-pipelined per unit as [ACT(n); sc(n+1);
filler; av(n)] so the scalar engine's exp stream (the true bottleneck,
~1.11 us/unit) never waits on PE, while projection work for later blocks is
dribbled in as 2 "filler" chunks per unit.  This also keeps instantaneous
PE duty below the chip activity governor's trip line (sustained ~>85% PE
duty triggers a k=4/n=8 duty throttle that halves the PE clock).
"""

import os
import sys

for _p in ("/opt/trn_rl_repo", "/root/.axon_site/_ro/trn_rl_repo"):
    if os.path.isdir(_p) and _p not in sys.path:
        sys.path.append(_p)

from collections import deque

import numpy as np

import concourse.bacc as bacc
import concourse.mybir as mybir
import concourse.tile as tile
from concourse.bass_utils import run_bass_kernel_spmd
from concourse.masks import make_identity

B = 4
T = 4096
C = 1024
H = 64
TQ = T // 2  # queries per core
N_CORES = 8

F32 = mybir.dt.float32
BF16 = mybir.dt.bfloat16

NC_CH = C // 128  # 8 contraction chunks
NSB = T // 512  # 8 key/source blocks of 512
NST = T // 128  # 32 key tiles of 128
NQB = TQ // 512  # 4 query blocks/chunks of 512


def _build_module():
    nc = bacc.Bacc("TRN2", target_bir_lowering=False, debug=False, num_devices=N_CORES)

    xt_d = nc.dram_tensor("xt", [2 * NSB, 128, 4, 512], BF16, kind="ExternalInput").ap()
    wkv_d = nc.dram_tensor("wkv", [128, NC_CH, 128], BF16, kind="ExternalInput").ap()
    wq_d = nc.dram_tensor("wq", [128, NC_CH, 64], BF16, kind="ExternalInput").ap()
    out = nc.dram_tensor("out", [NQB, 65, 512], F32, kind="ExternalOutput").ap()

    EXP = mybir.ActivationFunctionType.Exp

    with tile.TileContext(nc) as tc:
        with (
            tc.tile_pool(name="const", bufs=1) as const_pool,
            tc.tile_pool(name="xt", bufs=6) as xt_pool,
            tc.tile_pool(name="big", bufs=1) as big_pool,
            tc.tile_pool(name="vstage", bufs=2) as vstage_pool,
            tc.tile_pool(name="exp", bufs=5) as exp_pool,
            tc.tile_pool(name="outts", bufs=2) as outts_pool,
            tc.tile_pool(name="p1", bufs=2, space="PSUM") as psum_p1,
            tc.tile_pool(name="psc", bufs=2, space="PSUM") as psum_sc,
            tc.tile_pool(name="pacc", bufs=2, space="PSUM") as psum_acc,
        ):
            # ---- constants ----
            wkv_sb = const_pool.tile([128, NC_CH, 128], BF16, tag="wkv")
            wq_sb = const_pool.tile([128, NC_CH, 64], BF16, tag="wq")
            ident_bf = const_pool.tile([128, 128], BF16, tag="ident_bf")
            make_identity(nc, ident_bf[:])

            # ---- persistent activations ----
            kt_sb = big_pool.tile([64, T], BF16, tag="kt")  # K^T
            qt_sb = big_pool.tile([64, TQ], BF16, tag="qt")  # Q^T
            va = big_pool.tile([128, NST, 66], BF16, tag="va")  # V_aug per s-tile
            nc.gpsimd.memset(va[:, :, 64:65], 1.0)

            # scalar (the ACT engine's sequencer) is excluded: a DMA trigger
            # costs ~650ns of sequencer time and would hole the exp stream.
            dma_engines = (nc.sync, nc.gpsimd)
            dma_i = [0]

            def next_dma():
                e = dma_engines[dma_i[0] % len(dma_engines)]
                dma_i[0] += 1
                return e

            # ---- phase 1 pieces ----
            def emit_xdma(sb, quarters=False):
                xt = xt_pool.tile([128, NC_CH, 512], BF16, tag="xt", name=f"xt{sb}")
                if quarters:
                    for q in range(4):
                        next_dma().dma_start(
                            xt[:, 2 * q : 2 * q + 2, :],
                            xt_d[2 * sb + q // 2][:, 2 * (q % 2) : 2 * (q % 2) + 2, :],
                        )
                else:
                    for half in range(2):
                        next_dma().dma_start(
                            xt[:, 4 * half : 4 * half + 4, :], xt_d[2 * sb + half]
                        )
                return xt

            def emit_kv_mm(sb, xt, kv_ps, c0, c1):
                for c in range(c0, c1):
                    nc.tensor.matmul(
                        kv_ps[:],
                        wkv_sb[:, c, :],
                        xt[:, c, :],
                        start=(c == 0),
                        stop=(c == NC_CH - 1),
                    )

            def emit_kv_copies(sb, kv_ps):
                nc.vector.tensor_copy(kt_sb[:, sb * 512 : (sb + 1) * 512], kv_ps[0:64, :])
                vt_sb = vstage_pool.tile([128, 512], BF16, tag="vst", name=f"vt{sb}")
                nc.vector.tensor_copy(vt_sb[64:128, :], kv_ps[64:128, :])
                return vt_sb

            def emit_q_mm(sb, xt, q_ps, c0, c1):
                for c in range(c0, c1):
                    nc.tensor.matmul(
                        q_ps[:],
                        wq_sb[:, c, :],
                        xt[:, c, :],
                        start=(c == 0),
                        stop=(c == NC_CH - 1),
                    )

            def emit_v(sb, vt_sb, j0, j1):
                for j in range(j0, j1):  # V tiles of this block
                    st = sb * 4 + j
                    vt_ps = psum_p1.tile([128, 64], BF16, tag="p1", name=f"v{st}")
                    nc.tensor.transpose(
                        vt_ps[:],
                        vt_sb[64:128, j * 128 : (j + 1) * 128],
                        ident_bf[64:128, 64:128],
                    )
                    nc.vector.tensor_copy(va[:, st, 0:64], vt_ps[:])

            # filler queue: small PE chunks for proj blocks 1..7, drained
            # between attention units (~2 per unit).  KV/V pieces go first in
            # block order (attention consumes K/V tiles at 4 sts per block);
            # Q pieces for blocks 2,3 go at the tail (only needed when the
            # second query group starts at unit 32).
            filler = deque()
            proj_state = {}

            def queue_kv_block(sb, xt=None):
                state = proj_state.setdefault(sb, {})
                if xt is not None:
                    state["xt"] = xt

                def dma_piece(_sb=sb):
                    state["xt"] = emit_xdma(_sb)

                def kv_mm(c0, c1, _sb=sb):
                    if "kv" not in state:
                        state["kv"] = psum_p1.tile(
                            [128, 512], F32, tag="p1", name=f"kv{_sb}"
                        )
                    emit_kv_mm(_sb, state["xt"], state["kv"], c0, c1)

                def copies(_sb=sb):
                    state["vt"] = emit_kv_copies(_sb, state["kv"])

                if xt is None:
                    filler.append(dma_piece)
                for c0 in range(0, NC_CH, 2):
                    filler.append(lambda c0=c0: kv_mm(c0, c0 + 2))
                filler.append(copies)
                filler.append(lambda _sb=sb: emit_v(_sb, proj_state[_sb]["vt"], 0, 2))
                filler.append(lambda _sb=sb: emit_v(_sb, proj_state[_sb]["vt"], 2, 4))

            def queue_q_block(sb):
                state = proj_state[sb]

                def q_mm(c0, c1, _sb=sb):
                    if "q" not in state:
                        state["q"] = psum_p1.tile(
                            [64, 512], F32, tag="p1", name=f"q{_sb}"
                        )
                    emit_q_mm(_sb, state["xt"], state["q"], c0, c1)

                def q_copy(_sb=sb):
                    nc.vector.tensor_copy(
                        qt_sb[:, _sb * 512 : (_sb + 1) * 512], state["q"][:]
                    )

                for c0 in range(0, NC_CH, 2):
                    filler.append(lambda c0=c0: q_mm(c0, c0 + 2))
                filler.append(q_copy)

            # ---- phase 2: software-pipelined attention units ----
            # unit n = (tcp, st); per unit: ACT(n), sc(n+1), filler, av(n)
            units = [(0, st) for st in range(NST)] + [(1, st) for st in range(NST)]
            outt_tiles = {}
            sc_tiles = {}

            def emit_sc(n):
                tcp, st = units[n]
                sc_ps = psum_sc.tile([128, 1024], F32, tag="sc", name=f"sc{tcp}_{st}")
                sc_tiles[n] = sc_ps
                kt_slice = kt_sb[:, st * 128 : (st + 1) * 128]
                for i in range(2):
                    nc.tensor.matmul(
                        sc_ps[:, i * 512 : (i + 1) * 512],
                        kt_slice,
                        qt_sb[:, (2 * tcp + i) * 512 : (2 * tcp + i + 1) * 512],
                        start=True,
                        stop=True,
                    )

            def get_outt(tcp):
                if tcp not in outt_tiles:
                    oa = psum_acc.tile([65, 512], F32, tag="acc", name=f"outt_a{tcp}")
                    ob = psum_acc.tile([65, 512], F32, tag="acc", name=f"outt_b{tcp}")
                    outt_tiles[tcp] = (oa, ob)
                return outt_tiles[tcp]

            def emit_act_av(n, fillers_per_unit=2):
                tcp, st = units[n]
                sc_ps = sc_tiles.pop(n)
                ex = exp_pool.tile([128, 1024], BF16, tag="exp", name=f"ex{tcp}_{st}")
                nc.scalar.activation(ex[:], sc_ps[:], EXP, scale=0.125)
                if n + 1 < len(units):
                    emit_sc(n + 1)
                for _ in range(fillers_per_unit):
                    if filler:
                        filler.popleft()()
                for i, outt_ps in enumerate(get_outt(tcp)):
                    nc.tensor.matmul(
                        outt_ps[:],
                        va[:, st, 0:65],
                        ex[:, i * 512 : (i + 1) * 512],
                        start=(st == 0),
                        stop=(st == NST - 1),
                    )

            def emit_epilogue(tcp):
                for i, outt_ps in enumerate(outt_tiles[tcp]):
                    tci = 2 * tcp + i
                    outt_sb = outts_pool.tile([65, 512], F32, tag="outts", name=f"os{tci}")
                    nc.vector.tensor_copy(outt_sb[:], outt_ps[:])
                    nc.sync.dma_start(out[tci], outt_sb[:])

            # ---- emission ----
            # x block 0 + first weight chunk land first on every DMA queue so
            # the first projection group is never starved; x1/remaining
            # weights queue up behind them.
            xt0 = xt_pool.tile([128, NC_CH, 512], BF16, tag="xt", name="xt0")
            xt1 = xt_pool.tile([128, NC_CH, 512], BF16, tag="xt", name="xt1")

            def qdma(engine, dst, src):
                engine.dma_start(dst, src)

            qdma(nc.sync, xt0[:, 0:2, :], xt_d[0][:, 0:2, :])
            qdma(nc.gpsimd, xt0[:, 2:4, :], xt_d[0][:, 2:4, :])
            qdma(nc.scalar, xt0[:, 4:6, :], xt_d[1][:, 0:2, :])
            qdma(nc.sync, wkv_sb[:, 0:2, :], wkv_d[:, 0:2, :])
            qdma(nc.gpsimd, xt0[:, 6:8, :], xt_d[1][:, 2:4, :])
            qdma(nc.scalar, wkv_sb[:, 2:8, :], wkv_d[:, 2:8, :])
            qdma(nc.sync, wq_sb[:], wq_d)
            qdma(nc.gpsimd, xt1[:, 0:4, :], xt_d[2])
            qdma(nc.scalar, xt1[:, 4:8, :], xt_d[3])

            kv0 = psum_p1.tile([128, 512], F32, tag="p1", name="kv0")
            emit_kv_mm(0, xt0, kv0, 0, NC_CH)
            vt0 = emit_kv_copies(0, kv0)
            q0 = psum_p1.tile([64, 512], F32, tag="p1", name="q0")
            emit_q_mm(0, xt0, q0, 0, NC_CH)
            nc.vector.tensor_copy(qt_sb[:, 0:512], q0[:])
            q1 = psum_p1.tile([64, 512], F32, tag="p1", name="q1")
            emit_q_mm(1, xt1, q1, 0, NC_CH)
            nc.vector.tensor_copy(qt_sb[:, 512:1024], q1[:])
            # V tiles of block 0 + block 1 KV/V as the first fillers; then
            # blocks 2..7 KV/V, then the deferred Q projections for blocks 2,3.
            filler.append(lambda: emit_v(0, vt0, 0, 2))
            filler.append(lambda: emit_v(0, vt0, 2, 4))
            queue_kv_block(1, xt=xt1)
            for sb in range(2, NSB):
                queue_kv_block(sb)
            queue_q_block(2)
            queue_q_block(3)

            emit_sc(0)
            for n in range(len(units)):
                emit_act_av(n, fillers_per_unit=4 if n < 8 else 2)
                if n == NST - 1:
                    emit_epilogue(0)
            emit_epilogue(1)

    nc.compile()
    return nc


_NC_CACHE = None


def _get_module():
    global _NC_CACHE
    if _NC_CACHE is None:
        _NC_CACHE = _build_module()
    return _NC_CACHE


def _make_in_maps(x, Wq, Wk, Wv):
    import ml_dtypes

    bf16 = ml_dtypes.bfloat16
    xT = np.transpose(np.asarray(x, dtype=np.float32), (0, 2, 1)).astype(bf16)  # [B,C,T]
    wq = np.asarray(Wq, dtype=np.float32)
    wk = np.asarray(Wk, dtype=np.float32)
    wv = np.asarray(Wv, dtype=np.float32)
    wkv = (
        np.concatenate([wk, wv], axis=1)  # [1024, 128]
        .reshape(NC_CH, 128, 128)
        .transpose(1, 0, 2)
        .astype(bf16)
    )
    wqh = wq.reshape(NC_CH, 128, 64).transpose(1, 0, 2).astype(bf16)
    in_maps = []
    for core in range(N_CORES):
        b, h = divmod(core, 2)
        xb = xT[b]
        if h == 1:
            xb = np.concatenate([xb[:, TQ:], xb[:, :TQ]], axis=1)
        xb = (
            xb.reshape(2, 4, 128, NSB, 512)
            .transpose(3, 0, 2, 1, 4)
            .reshape(2 * NSB, 128, 4, 512)
        )
        in_maps.append(
            {
                "xt": np.ascontiguousarray(xb),
                "wkv": np.ascontiguousarray(wkv),
                "wq": np.ascontiguousarray(wqh),
            }
        )
    return in_maps


def _unshard(results):
    out = np.empty((B, T, H), dtype=np.float32)
    for core in range(N_CORES):
        b, h = divmod(core, 2)
        slab = results[core]["out"]  # [NQB, 65, 512]
        num = slab[:, 0:64, :]
        den = slab[:, 64:65, :]
        o = (num / den).transpose(0, 2, 1).reshape(TQ, H)
        out[b, h * TQ : (h + 1) * TQ, :] = o
    return out


def run(x, Wq, Wk, Wv, **spmd_kwargs):
    """Run on hardware; returns (output, BassKernelResults)."""
    nc = _get_module()
    in_maps = _make_in_maps(x, Wq, Wk, Wv)
    res = run_bass_kernel_spmd(nc, in_maps, core_ids=list(range(N_CORES)), **spmd_kwargs)
    return _unshard(res.results), res


def kernel(x, Wq, Wk, Wv):
    out, _ = run(x, Wq, Wk, Wv)
    return out
